# revision 32
# baseline (speedup 1.0000x reference)
"""Sliding-window causal attention (B=2, H=16, T=2048, D=64, WINDOW=512) on
8 TRN2 NeuronCores.

Sharding: the 32 (b, h) pairs are split 4-per-core (embarrassingly parallel).
Each core runs the same Bass/Tile program over its 4 heads.

Per-head algorithm (all on one core):
  - Q, K are transposed on-chip to d-major layout ([64, T]) with PE
    transposes; two heads are packed per [128, 128] transpose.
  - For each 128-wide key block kb, compute S^T[k, q] = Kd^T @ Qd over the
    query span [128*kb, 128*kb + 640) (sliding window 512 + causal).
  - exp(scale * S^T) runs on the scalar engine straight out of PSUM into a
    bf16 E^T tile; invalid triangles of the two boundary sub-tiles are
    zeroed with gpsimd affine_select.  No max-subtraction: scores are
    ~N(0, 1) after scaling, exp is safe in fp32.
  - PV: O^T[65, q] accumulates in PSUM via bf16 matmuls with stationary
    V' = [V | ones]; row 64 collects the softmax denominator.
  - Drain per 4 query blocks: DVE reciprocal of the denominator column,
    broadcast multiply, one batched DMA out.
"""

import sys
from contextlib import ExitStack

import numpy as np

sys.path.insert(0, "/opt/trn_rl_repo")

import concourse.bacc as bacc
import concourse.tile as tile
from concourse import mybir
from concourse.bass_utils import run_bass_kernel_spmd

F32 = mybir.dt.float32
BF16 = mybir.dt.bfloat16
EXP = mybir.ActivationFunctionType.Exp

B, H, T, D = 2, 16, 2048, 64
WINDOW = 512
SCALE = D ** -0.5
N_CORES = 8
HEADS_PER_CORE = (B * H) // N_CORES  # 4
TB = T // 128  # 16 query/key blocks


def build_nc(t=T, heads_per_core=HEADS_PER_CORE):
    nb = t // 128  # number of 128-blocks along the sequence

    nc = bacc.Bacc("TRN2", target_bir_lowering=False)
    q_ext = nc.declare_dram_parameter("q", [heads_per_core, t, D], F32, isOutput=False)
    k_ext = nc.declare_dram_parameter("k", [heads_per_core, t, D], F32, isOutput=False)
    v_ext = nc.declare_dram_parameter("v", [heads_per_core, t, D], F32, isOutput=False)
    id_ext = nc.declare_dram_parameter("ident", [128, 128], F32, isOutput=False)
    o_ext = nc.declare_dram_parameter("out", [heads_per_core, t, D], F32, isOutput=True)

    assert heads_per_core % 2 == 0

    with tile.TileContext(nc) as tc, ExitStack() as ctx:
        const = ctx.enter_context(tc.tile_pool(name="const", bufs=1))
        stage = ctx.enter_context(tc.tile_pool(name="stage", bufs=2))
        qkd = ctx.enter_context(tc.tile_pool(name="qkd", bufs=2))
        vps = ctx.enter_context(tc.tile_pool(name="vps", bufs=3))
        ets = ctx.enter_context(tc.tile_pool(name="ets", bufs=13))
        outs = ctx.enter_context(tc.tile_pool(name="outs", bufs=3))
        tr_ps = ctx.enter_context(tc.tile_pool(name="tr_ps", bufs=1, space="PSUM"))
        s_ps = ctx.enter_context(tc.tile_pool(name="s_ps", bufs=3, space="PSUM"))
        ob_ps = ctx.enter_context(tc.tile_pool(name="ob_ps", bufs=1, space="PSUM"))

        # fp32 identity (for fp32 O^T transposes) + bf16 copy (for Q/K).
        ident_f = const.tile([128, 128], F32, tag="ident_f")
        nc.sync.dma_start(out=ident_f[:], in_=id_ext[:])
        ident_b = const.tile([128, 128], BF16, tag="ident_b")
        nc.vector.tensor_copy(ident_b[:], ident_f[:])

        # multiplicative mask for E^T tiles: cols 0:128 keep c >= r (causal
        # diagonal), cols 128:512 all-ones, cols 512:640 keep c < r (window).
        mask = const.tile([128, 640], BF16, tag="mask")
        nc.gpsimd.memset(mask[:, 0:512], 1.0)
        nc.gpsimd.affine_select(
            out=mask[:, 0:128],
            in_=mask[:, 0:128],
            compare_op=mybir.AluOpType.is_ge,
            fill=0.0,
            base=0,
            pattern=[[1, 128]],
            channel_multiplier=-1,
        )
        nc.gpsimd.memset(mask[:, 512:640], 1.0)
        nc.gpsimd.affine_select(
            out=mask[:, 512:640],
            in_=mask[:, 512:640],
            compare_op=mybir.AluOpType.is_ge,
            fill=0.0,
            base=-1,
            pattern=[[-1, 128]],
            channel_multiplier=1,
        )

        for pair in range(heads_per_core // 2):
            hA, hB = 2 * pair, 2 * pair + 1

            # ---- Q/K -> d-major bf16 [128, t]; rows 0:64 head A, 64:128
            # head B.  One whole-tensor DMA per (tensor, head) and one big
            # cast: staging previously issued 24 small DMAs per pair, whose
            # serialized issue dominated the kernel prologue.
            qd = qkd.tile([128, t], BF16, tag="qd")
            kd = qkd.tile([128, t], BF16, tag="kd")
            for ext, dst in ((q_ext, qd), (k_ext, kd)):
                st_f = stage.tile([128, t], F32, tag="st_f")
                st3 = st_f[:].rearrange("p (b c) -> p b c", c=128)
                for hh, doff in ((hA, 0), (hB, 64)):
                    nc.sync.dma_start(
                        out=st3[:, :, doff : doff + 64],
                        in_=ext[hh, :, :].rearrange("(b p) d -> p b d", p=128),
                    )
                st_b = stage.tile([128, t], BF16, tag="st_b")
                nc.vector.tensor_copy(st_b[:], st_f[:])
                for b4 in range(nb // 4):
                    trp = tr_ps.tile([128, 512], BF16, tag="trp")
                    for i in range(4):
                        tb = 4 * b4 + i
                        nc.tensor.transpose(
                            trp[:, i * 128 : (i + 1) * 128],
                            st_b[:, tb * 128 : (tb + 1) * 128],
                            ident_b[:],
                        )
                    nc.vector.tensor_copy(dst[:, b4 * 512 : (b4 + 1) * 512], trp[:])

            # ---- V' = [V | 1] bf16 per head: [128, nb, 65]
            vp = {}
            for h in (hA, hB):
                vt = vps.tile([128, nb, 65], BF16, tag="vp")
                st_f = stage.tile([128, t], F32, tag="st_f")
                nc.sync.dma_start(
                    out=st_f[:].rearrange("p (b c) -> p b c", c=128)[:, :, 0:64],
                    in_=v_ext[h, :, :].rearrange("(b p) d -> p b d", p=128),
                )
                nc.vector.tensor_copy(
                    vt[:, :, 0:64],
                    st_f[:].rearrange("p (b d) -> p b d", b=nb)[:, :, 0:64],
                )
                nc.vector.memset(vt[:, :, 64:65], 1.0)
                vp[h] = vt

            # ---- attention, the two heads of the pair interleaved so the
            # PE always has a second independent stream (keeps the systolic
            # array busy while exp/mask of the other head run).  PV uses the
            # E^T block as the stationary operand and V' as moving, directly
            # producing O[q, d] + denominator (col 64) -- no O transpose.
            rows_of = {hA: slice(0, 64), hB: slice(64, 128)}
            et = {hA: {}, hB: {}}
            of = {}

            def produce_et(kb, h):
                rows = rows_of[h]
                span = min(640, t - kb * 128)
                e = ets.tile([128, 640], BF16, tag="et", name=f"et_{h}_{kb}")
                et[h][kb] = e
                sp = s_ps.tile([128, 640], F32, tag="s", name=f"sp_{h}_{kb}")
                off = 0
                while off < span:
                    n = min(512, span - off)
                    nc.tensor.matmul(
                        sp[:, off : off + n],
                        kd[rows, kb * 128 : (kb + 1) * 128],
                        qd[rows, kb * 128 + off : kb * 128 + off + n],
                        start=True,
                        stop=True,
                    )
                    off += n
                nc.scalar.activation(e[:, 0:span], sp[:, 0:span], EXP, scale=SCALE)
                # zero masked triangles (causal diagonal + window boundary)
                nc.vector.tensor_mul(e[:, 0:128], e[:, 0:128], mask[:, 0:128])
                if span == 640:
                    nc.vector.tensor_mul(
                        e[:, 512:640], e[:, 512:640], mask[:, 512:640]
                    )

            for h in (hA, hB):
                produce_et(0, h)
                of = None
                for qb in range(nb):
                    if qb + 1 < nb:
                        produce_et(qb + 1, h)
                    # PV: O[q, :64] + denominator in col 64, serial accumulation
                    ob = ob_ps.tile([128, 65], F32, tag="ob")
                    kb0 = max(0, qb - 4)
                    for kb in range(kb0, qb + 1):
                        nc.tensor.matmul(
                            ob[:],
                            et[h][kb][:, (qb - kb) * 128 : (qb - kb) * 128 + 128],
                            vp[h][:, kb, :],
                            start=(kb == kb0),
                            stop=(kb == qb),
                        )
                    if qb >= 4:
                        del et[h][qb - 4]
                    # drain + normalize, batched per 4 query blocks
                    if qb % 4 == 0:
                        of = outs.tile([128, 4 * 65], F32, tag="of", name=f"of_{h}_{qb}")
                    nc.vector.tensor_copy(of[:, (qb % 4) * 65 : (qb % 4) * 65 + 65], ob[:])
                    if qb % 4 == 3 or qb == nb - 1:
                        g = qb // 4
                        nq = qb % 4 + 1
                        rc = outs.tile([128, 4], F32, tag="rc")
                        of3 = of[:].rearrange("p (b c) -> p b c", c=65)
                        nc.vector.reciprocal(rc[:, 0:nq], of3[:, 0:nq, 64])
                        oo = outs.tile([128, 4 * 64], F32, tag="oo")
                        nc.vector.tensor_mul(
                            oo[:, 0 : nq * 64].rearrange("p (b c) -> p b c", c=64),
                            of3[:, 0:nq, 0:64],
                            rc[:, 0:nq].rearrange("p (b c) -> p b c", c=1).broadcast_to(
                                [128, nq, 64]
                            ),
                        )
                        nc.sync.dma_start(
                            out=o_ext[
                                h, g * 512 : g * 512 + nq * 128, :
                            ].rearrange("(b p) d -> p b d", p=128),
                            in_=oo[:, 0 : nq * 64].rearrange(
                                "p (b c) -> p b c", c=64
                            ),
                        )

    nc.compile()
    return nc


_NC_CACHE = {}
TRACE = False
TRACE_DIR = None
LAST_RESULT = None


def _get_nc():
    key = (T, HEADS_PER_CORE)
    if key not in _NC_CACHE:
        _NC_CACHE[key] = build_nc()
    return _NC_CACHE[key]


def kernel(q, k, v):
    q = np.ascontiguousarray(np.asarray(q, dtype=np.float32))
    k = np.ascontiguousarray(np.asarray(k, dtype=np.float32))
    v = np.ascontiguousarray(np.asarray(v, dtype=np.float32))
    assert q.shape == (B, H, T, D)

    qf = q.reshape(B * H, T, D)
    kf = k.reshape(B * H, T, D)
    vf = v.reshape(B * H, T, D)
    ident = np.eye(128, dtype=np.float32)

    in_maps = []
    for c in range(N_CORES):
        s = slice(c * HEADS_PER_CORE, (c + 1) * HEADS_PER_CORE)
        in_maps.append(
            {
                "q": np.ascontiguousarray(qf[s]),
                "k": np.ascontiguousarray(kf[s]),
                "v": np.ascontiguousarray(vf[s]),
                "ident": ident,
            }
        )

    nc = _get_nc()
    global LAST_RESULT
    res = run_bass_kernel_spmd(
        nc, in_maps, list(range(N_CORES)), trace=TRACE, tmpdir=TRACE_DIR
    )
    LAST_RESULT = res
    out = np.concatenate([res.results[c]["out"] for c in range(N_CORES)], axis=0)
    return out.reshape(B, H, T, D).astype(np.float32)


# revision 33
# speedup vs baseline: 1.0377x; 1.0377x over previous
"""Sliding-window causal attention (B=2, H=16, T=2048, D=64, WINDOW=512) on
8 TRN2 NeuronCores.

Sharding: the 32 (b, h) pairs are split 4-per-core (embarrassingly parallel).
Each core runs the same Bass/Tile program over its 4 heads.

Per-head algorithm (all on one core):
  - Q, K are transposed on-chip to d-major layout ([64, T]) with PE
    transposes; two heads are packed per [128, 128] transpose.
  - For each 128-wide key block kb, compute S^T[k, q] = Kd^T @ Qd over the
    query span [128*kb, 128*kb + 640) (sliding window 512 + causal).
  - exp(scale * S^T) runs on the scalar engine straight out of PSUM into a
    bf16 E^T tile; invalid triangles of the two boundary sub-tiles are
    zeroed with gpsimd affine_select.  No max-subtraction: scores are
    ~N(0, 1) after scaling, exp is safe in fp32.
  - PV: O^T[65, q] accumulates in PSUM via bf16 matmuls with stationary
    V' = [V | ones]; row 64 collects the softmax denominator.
  - Drain per 4 query blocks: DVE reciprocal of the denominator column,
    broadcast multiply, one batched DMA out.
"""

import sys
from contextlib import ExitStack

import numpy as np

sys.path.insert(0, "/opt/trn_rl_repo")

import concourse.bacc as bacc
import concourse.tile as tile
from concourse import mybir
from concourse.bass_utils import run_bass_kernel_spmd

F32 = mybir.dt.float32
BF16 = mybir.dt.bfloat16
EXP = mybir.ActivationFunctionType.Exp

B, H, T, D = 2, 16, 2048, 64
WINDOW = 512
SCALE = D ** -0.5
N_CORES = 8
HEADS_PER_CORE = (B * H) // N_CORES  # 4
TB = T // 128  # 16 query/key blocks


def build_nc(t=T, heads_per_core=HEADS_PER_CORE):
    nb = t // 128  # number of 128-blocks along the sequence

    nc = bacc.Bacc("TRN2", target_bir_lowering=False)
    q_ext = nc.declare_dram_parameter("q", [heads_per_core, t, D], F32, isOutput=False)
    k_ext = nc.declare_dram_parameter("k", [heads_per_core, t, D], F32, isOutput=False)
    v_ext = nc.declare_dram_parameter("v", [heads_per_core, t, D], F32, isOutput=False)
    id_ext = nc.declare_dram_parameter("ident", [128, 128], F32, isOutput=False)
    o_ext = nc.declare_dram_parameter("out", [heads_per_core, t, D], F32, isOutput=True)

    assert heads_per_core % 2 == 0

    with tile.TileContext(nc) as tc, ExitStack() as ctx:
        const = ctx.enter_context(tc.tile_pool(name="const", bufs=1))
        stage = ctx.enter_context(tc.tile_pool(name="stage", bufs=6))
        qkd = ctx.enter_context(tc.tile_pool(name="qkd", bufs=2))
        vps = ctx.enter_context(tc.tile_pool(name="vps", bufs=3))
        ets = ctx.enter_context(tc.tile_pool(name="ets", bufs=13))
        outs = ctx.enter_context(tc.tile_pool(name="outs", bufs=3))
        tr_ps = ctx.enter_context(tc.tile_pool(name="tr_ps", bufs=1, space="PSUM"))
        s_ps = ctx.enter_context(tc.tile_pool(name="s_ps", bufs=3, space="PSUM"))
        ob_ps = ctx.enter_context(tc.tile_pool(name="ob_ps", bufs=1, space="PSUM"))

        # fp32 identity (for fp32 O^T transposes) + bf16 copy (for Q/K).
        ident_f = const.tile([128, 128], F32, tag="ident_f")
        nc.sync.dma_start(out=ident_f[:], in_=id_ext[:])
        ident_b = const.tile([128, 128], BF16, tag="ident_b")
        nc.vector.tensor_copy(ident_b[:], ident_f[:])

        # multiplicative mask for E^T tiles: cols 0:128 keep c >= r (causal
        # diagonal), cols 128:512 all-ones, cols 512:640 keep c < r (window).
        mask = const.tile([128, 640], BF16, tag="mask")
        nc.gpsimd.memset(mask[:, 0:512], 1.0)
        nc.gpsimd.affine_select(
            out=mask[:, 0:128],
            in_=mask[:, 0:128],
            compare_op=mybir.AluOpType.is_ge,
            fill=0.0,
            base=0,
            pattern=[[1, 128]],
            channel_multiplier=-1,
        )
        nc.gpsimd.memset(mask[:, 512:640], 1.0)
        nc.gpsimd.affine_select(
            out=mask[:, 512:640],
            in_=mask[:, 512:640],
            compare_op=mybir.AluOpType.is_ge,
            fill=0.0,
            base=-1,
            pattern=[[-1, 128]],
            channel_multiplier=1,
        )

        for pair in range(heads_per_core // 2):
            hA, hB = 2 * pair, 2 * pair + 1

            # ---- Q/K -> d-major bf16 [128, t]; rows 0:64 head A, 64:128 head B
            qd = qkd.tile([128, t], BF16, tag="qd")
            kd = qkd.tile([128, t], BF16, tag="kd")
            for ext, dst in ((q_ext, qd), (k_ext, kd)):
                for b4 in range(nb // 4):
                    st_f = stage.tile([128, 512], F32, tag="st_f")
                    rows = slice(b4 * 512, (b4 + 1) * 512)
                    st3 = st_f[:].rearrange("p (b c) -> p b c", c=128)
                    for hh, doff in ((hA, 0), (hB, 64)):
                        nc.sync.dma_start(
                            out=st3[:, :, doff : doff + 64],
                            in_=ext[hh, rows, :].rearrange("(b p) d -> p b d", p=128),
                        )
                    st_b = stage.tile([128, 512], BF16, tag="st_b")
                    nc.vector.tensor_copy(st_b[:], st_f[:])
                    trp = tr_ps.tile([128, 512], BF16, tag="trp")
                    for i in range(4):
                        nc.tensor.transpose(
                            trp[:, i * 128 : (i + 1) * 128],
                            st_b[:, i * 128 : (i + 1) * 128],
                            ident_b[:],
                        )
                    nc.vector.tensor_copy(dst[:, b4 * 512 : (b4 + 1) * 512], trp[:])

            # ---- V' = [V | 1] bf16 per head: [128, nb, 65]
            vp = {}
            for h in (hA, hB):
                vt = vps.tile([128, nb, 65], BF16, tag="vp")
                for b4 in range(nb // 4):
                    st_f = stage.tile([128, 512], F32, tag="st_f")
                    nc.sync.dma_start(
                        out=st_f[:].rearrange("p (b c) -> p b c", c=128)[:, :, 0:64],
                        in_=v_ext[h, b4 * 512 : (b4 + 1) * 512, :].rearrange(
                            "(b p) d -> p b d", p=128
                        ),
                    )
                    nc.vector.tensor_copy(
                        vt[:, 4 * b4 : 4 * b4 + 4, 0:64],
                        st_f[:].rearrange("p (b d) -> p b d", b=4)[:, :, 0:64],
                    )
                nc.vector.memset(vt[:, :, 64:65], 1.0)
                vp[h] = vt

            # ---- attention, the two heads of the pair interleaved so the
            # PE always has a second independent stream (keeps the systolic
            # array busy while exp/mask of the other head run).  PV uses the
            # E^T block as the stationary operand and V' as moving, directly
            # producing O[q, d] + denominator (col 64) -- no O transpose.
            rows_of = {hA: slice(0, 64), hB: slice(64, 128)}
            et = {hA: {}, hB: {}}
            of = {}

            def produce_et(kb, h):
                rows = rows_of[h]
                span = min(640, t - kb * 128)
                e = ets.tile([128, 640], BF16, tag="et", name=f"et_{h}_{kb}")
                et[h][kb] = e
                sp = s_ps.tile([128, 640], F32, tag="s", name=f"sp_{h}_{kb}")
                off = 0
                while off < span:
                    n = min(512, span - off)
                    nc.tensor.matmul(
                        sp[:, off : off + n],
                        kd[rows, kb * 128 : (kb + 1) * 128],
                        qd[rows, kb * 128 + off : kb * 128 + off + n],
                        start=True,
                        stop=True,
                    )
                    off += n
                nc.scalar.activation(e[:, 0:span], sp[:, 0:span], EXP, scale=SCALE)
                # zero masked triangles in place on gpsimd (otherwise idle)
                nc.gpsimd.affine_select(
                    out=e[:, 0:128],
                    in_=e[:, 0:128],
                    compare_op=mybir.AluOpType.is_ge,
                    fill=0.0,
                    base=0,
                    pattern=[[1, 128]],
                    channel_multiplier=-1,
                )
                if span == 640:
                    nc.gpsimd.affine_select(
                        out=e[:, 512:640],
                        in_=e[:, 512:640],
                        compare_op=mybir.AluOpType.is_ge,
                        fill=0.0,
                        base=-1,
                        pattern=[[-1, 128]],
                        channel_multiplier=1,
                    )

            for h in (hA, hB):
                produce_et(0, h)
                of = None
                for qb in range(nb):
                    if qb + 1 < nb:
                        produce_et(qb + 1, h)
                    # PV: O[q, :64] + denominator in col 64, serial accumulation
                    ob = ob_ps.tile([128, 65], F32, tag="ob")
                    kb0 = max(0, qb - 4)
                    for kb in range(kb0, qb + 1):
                        nc.tensor.matmul(
                            ob[:],
                            et[h][kb][:, (qb - kb) * 128 : (qb - kb) * 128 + 128],
                            vp[h][:, kb, :],
                            start=(kb == kb0),
                            stop=(kb == qb),
                        )
                    if qb >= 4:
                        del et[h][qb - 4]
                    # drain + normalize, batched per 4 query blocks
                    if qb % 4 == 0:
                        of = outs.tile([128, 4 * 65], F32, tag="of", name=f"of_{h}_{qb}")
                    nc.vector.tensor_copy(of[:, (qb % 4) * 65 : (qb % 4) * 65 + 65], ob[:])
                    if qb % 4 == 3 or qb == nb - 1:
                        g = qb // 4
                        nq = qb % 4 + 1
                        rc = outs.tile([128, 4], F32, tag="rc")
                        of3 = of[:].rearrange("p (b c) -> p b c", c=65)
                        nc.vector.reciprocal(rc[:, 0:nq], of3[:, 0:nq, 64])
                        oo = outs.tile([128, 4 * 64], F32, tag="oo")
                        nc.vector.tensor_mul(
                            oo[:, 0 : nq * 64].rearrange("p (b c) -> p b c", c=64),
                            of3[:, 0:nq, 0:64],
                            rc[:, 0:nq].rearrange("p (b c) -> p b c", c=1).broadcast_to(
                                [128, nq, 64]
                            ),
                        )
                        nc.sync.dma_start(
                            out=o_ext[
                                h, g * 512 : g * 512 + nq * 128, :
                            ].rearrange("(b p) d -> p b d", p=128),
                            in_=oo[:, 0 : nq * 64].rearrange(
                                "p (b c) -> p b c", c=64
                            ),
                        )

    nc.compile()
    return nc


_NC_CACHE = {}
TRACE = False
TRACE_DIR = None
LAST_RESULT = None


def _get_nc():
    key = (T, HEADS_PER_CORE)
    if key not in _NC_CACHE:
        _NC_CACHE[key] = build_nc()
    return _NC_CACHE[key]


def kernel(q, k, v):
    q = np.ascontiguousarray(np.asarray(q, dtype=np.float32))
    k = np.ascontiguousarray(np.asarray(k, dtype=np.float32))
    v = np.ascontiguousarray(np.asarray(v, dtype=np.float32))
    assert q.shape == (B, H, T, D)

    qf = q.reshape(B * H, T, D)
    kf = k.reshape(B * H, T, D)
    vf = v.reshape(B * H, T, D)
    ident = np.eye(128, dtype=np.float32)

    in_maps = []
    for c in range(N_CORES):
        s = slice(c * HEADS_PER_CORE, (c + 1) * HEADS_PER_CORE)
        in_maps.append(
            {
                "q": np.ascontiguousarray(qf[s]),
                "k": np.ascontiguousarray(kf[s]),
                "v": np.ascontiguousarray(vf[s]),
                "ident": ident,
            }
        )

    nc = _get_nc()
    global LAST_RESULT
    res = run_bass_kernel_spmd(
        nc, in_maps, list(range(N_CORES)), trace=TRACE, tmpdir=TRACE_DIR
    )
    LAST_RESULT = res
    out = np.concatenate([res.results[c]["out"] for c in range(N_CORES)], axis=0)
    return out.reshape(B, H, T, D).astype(np.float32)


# revision 34
# speedup vs baseline: 1.0430x; 1.0051x over previous
"""Sliding-window causal attention (B=2, H=16, T=2048, D=64, WINDOW=512) on
8 TRN2 NeuronCores.

Sharding: the 32 (b, h) pairs are split 4-per-core (embarrassingly parallel).
Each core runs the same Bass/Tile program over its 4 heads.

Per-head algorithm (all on one core):
  - Q, K are transposed on-chip to d-major layout ([64, T]) with PE
    transposes; two heads are packed per [128, 128] transpose.
  - For each 128-wide key block kb, compute S^T[k, q] = Kd^T @ Qd over the
    query span [128*kb, 128*kb + 640) (sliding window 512 + causal).
  - exp(scale * S^T) runs on the scalar engine straight out of PSUM into a
    bf16 E^T tile; invalid triangles of the two boundary sub-tiles are
    zeroed with gpsimd affine_select.  No max-subtraction: scores are
    ~N(0, 1) after scaling, exp is safe in fp32.
  - PV: O^T[65, q] accumulates in PSUM via bf16 matmuls with stationary
    V' = [V | ones]; row 64 collects the softmax denominator.
  - Drain per 4 query blocks: DVE reciprocal of the denominator column,
    broadcast multiply, one batched DMA out.
"""

import sys
from contextlib import ExitStack

import numpy as np

sys.path.insert(0, "/opt/trn_rl_repo")

import concourse.bacc as bacc
import concourse.tile as tile
from concourse import mybir
from concourse.bass_utils import run_bass_kernel_spmd

F32 = mybir.dt.float32
BF16 = mybir.dt.bfloat16
EXP = mybir.ActivationFunctionType.Exp

B, H, T, D = 2, 16, 2048, 64
WINDOW = 512
SCALE = D ** -0.5
N_CORES = 8
HEADS_PER_CORE = (B * H) // N_CORES  # 4
TB = T // 128  # 16 query/key blocks


def build_nc(t=T, heads_per_core=HEADS_PER_CORE):
    nb = t // 128  # number of 128-blocks along the sequence

    nc = bacc.Bacc("TRN2", target_bir_lowering=False)
    q_ext = nc.declare_dram_parameter("q", [heads_per_core, t, D], F32, isOutput=False)
    k_ext = nc.declare_dram_parameter("k", [heads_per_core, t, D], F32, isOutput=False)
    v_ext = nc.declare_dram_parameter("v", [heads_per_core, t, D], F32, isOutput=False)
    id_ext = nc.declare_dram_parameter("ident", [128, 128], F32, isOutput=False)
    o_ext = nc.declare_dram_parameter("out", [heads_per_core, t, D], F32, isOutput=True)

    assert heads_per_core % 2 == 0

    with tile.TileContext(nc) as tc, ExitStack() as ctx:
        const = ctx.enter_context(tc.tile_pool(name="const", bufs=1))
        stage = ctx.enter_context(tc.tile_pool(name="stage", bufs=6))
        qkd = ctx.enter_context(tc.tile_pool(name="qkd", bufs=2))
        vps = ctx.enter_context(tc.tile_pool(name="vps", bufs=3))
        ets = ctx.enter_context(tc.tile_pool(name="ets", bufs=13))
        outs = ctx.enter_context(tc.tile_pool(name="outs", bufs=3))
        tr_ps = ctx.enter_context(tc.tile_pool(name="tr_ps", bufs=1, space="PSUM"))
        s_ps = ctx.enter_context(tc.tile_pool(name="s_ps", bufs=3, space="PSUM"))
        ob_ps = ctx.enter_context(tc.tile_pool(name="ob_ps", bufs=1, space="PSUM"))

        # fp32 identity (for fp32 O^T transposes) + bf16 copy (for Q/K).
        ident_f = const.tile([128, 128], F32, tag="ident_f")
        nc.sync.dma_start(out=ident_f[:], in_=id_ext[:])
        ident_b = const.tile([128, 128], BF16, tag="ident_b")
        nc.vector.tensor_copy(ident_b[:], ident_f[:])

        # multiplicative mask for E^T tiles: cols 0:128 keep c >= r (causal
        # diagonal), cols 128:512 all-ones, cols 512:640 keep c < r (window).
        mask = const.tile([128, 640], BF16, tag="mask")
        nc.gpsimd.memset(mask[:, 0:512], 1.0)
        nc.gpsimd.affine_select(
            out=mask[:, 0:128],
            in_=mask[:, 0:128],
            compare_op=mybir.AluOpType.is_ge,
            fill=0.0,
            base=0,
            pattern=[[1, 128]],
            channel_multiplier=-1,
        )
        nc.gpsimd.memset(mask[:, 512:640], 1.0)
        nc.gpsimd.affine_select(
            out=mask[:, 512:640],
            in_=mask[:, 512:640],
            compare_op=mybir.AluOpType.is_ge,
            fill=0.0,
            base=-1,
            pattern=[[-1, 128]],
            channel_multiplier=1,
        )

        for pair in range(heads_per_core // 2):
            hA, hB = 2 * pair, 2 * pair + 1

            # ---- Q/K -> d-major bf16, halved along the sequence and staged
            # in order (Q-lo, K-lo, Q-hi, K-hi) so the first QK matmuls only
            # wait on the first two staging units.  rows 0:64 head A,
            # 64:128 head B.
            th = max(t // 2, 512)
            qd_halves = [
                qkd.tile([128, th], BF16, tag="qd0", name=f"qd0_{pair}"),
                qkd.tile([128, th], BF16, tag="qd1", name=f"qd1_{pair}"),
            ]
            kd_halves = [
                qkd.tile([128, th], BF16, tag="kd0", name=f"kd0_{pair}"),
                qkd.tile([128, th], BF16, tag="kd1", name=f"kd1_{pair}"),
            ]
            batches = []
            for half in range(t // th):
                for ext, halves in ((q_ext, qd_halves), (k_ext, kd_halves)):
                    for b4 in range(half * th // 512, (half + 1) * th // 512):
                        batches.append((ext, halves, b4))
            for ext, halves, b4 in batches:
                if True:
                    dst = halves[(b4 * 512) // th]
                    dcol = (b4 * 512) % th
                    st_f = stage.tile([128, 512], F32, tag="st_f")
                    rows = slice(b4 * 512, (b4 + 1) * 512)
                    st3 = st_f[:].rearrange("p (b c) -> p b c", c=128)
                    for hh, doff in ((hA, 0), (hB, 64)):
                        nc.sync.dma_start(
                            out=st3[:, :, doff : doff + 64],
                            in_=ext[hh, rows, :].rearrange("(b p) d -> p b d", p=128),
                        )
                    st_b = stage.tile([128, 512], BF16, tag="st_b")
                    nc.vector.tensor_copy(st_b[:], st_f[:])
                    trp = tr_ps.tile([128, 512], BF16, tag="trp")
                    for i in range(4):
                        nc.tensor.transpose(
                            trp[:, i * 128 : (i + 1) * 128],
                            st_b[:, i * 128 : (i + 1) * 128],
                            ident_b[:],
                        )
                    nc.vector.tensor_copy(dst[:, dcol : dcol + 512], trp[:])

            # ---- V' = [V | 1] bf16 per head: [128, nb, 65]
            vp = {}
            for h in (hA, hB):
                vt = vps.tile([128, nb, 65], BF16, tag="vp")
                for b4 in range(nb // 4):
                    st_f = stage.tile([128, 512], F32, tag="st_f")
                    nc.sync.dma_start(
                        out=st_f[:].rearrange("p (b c) -> p b c", c=128)[:, :, 0:64],
                        in_=v_ext[h, b4 * 512 : (b4 + 1) * 512, :].rearrange(
                            "(b p) d -> p b d", p=128
                        ),
                    )
                    nc.vector.tensor_copy(
                        vt[:, 4 * b4 : 4 * b4 + 4, 0:64],
                        st_f[:].rearrange("p (b d) -> p b d", b=4)[:, :, 0:64],
                    )
                nc.vector.memset(vt[:, :, 64:65], 1.0)
                vp[h] = vt

            # ---- attention, the two heads of the pair interleaved so the
            # PE always has a second independent stream (keeps the systolic
            # array busy while exp/mask of the other head run).  PV uses the
            # E^T block as the stationary operand and V' as moving, directly
            # producing O[q, d] + denominator (col 64) -- no O transpose.
            rows_of = {hA: slice(0, 64), hB: slice(64, 128)}
            et = {hA: {}, hB: {}}
            of = {}

            def produce_et(kb, h):
                rows = rows_of[h]
                span = min(640, t - kb * 128)
                e = ets.tile([128, 640], BF16, tag="et", name=f"et_{h}_{kb}")
                et[h][kb] = e
                sp = s_ps.tile([128, 640], F32, tag="s", name=f"sp_{h}_{kb}")
                kdt = kd_halves[(kb * 128) // th]
                kcol = (kb * 128) % th
                a = kb * 128
                for lo in range(0, t, th):
                    s0, s1 = max(a, lo), min(a + span, lo + th)
                    while s0 < s1:
                        n = min(512 - (s0 - a) % 512, s1 - s0)
                        nc.tensor.matmul(
                            sp[:, s0 - a : s0 - a + n],
                            kdt[rows, kcol : kcol + 128],
                            qd_halves[lo // th][rows, s0 - lo : s0 - lo + n],
                            start=True,
                            stop=True,
                        )
                        s0 += n
                nc.scalar.activation(e[:, 0:span], sp[:, 0:span], EXP, scale=SCALE)
                # zero masked triangles in place on gpsimd (otherwise idle)
                nc.gpsimd.affine_select(
                    out=e[:, 0:128],
                    in_=e[:, 0:128],
                    compare_op=mybir.AluOpType.is_ge,
                    fill=0.0,
                    base=0,
                    pattern=[[1, 128]],
                    channel_multiplier=-1,
                )
                if span == 640:
                    nc.gpsimd.affine_select(
                        out=e[:, 512:640],
                        in_=e[:, 512:640],
                        compare_op=mybir.AluOpType.is_ge,
                        fill=0.0,
                        base=-1,
                        pattern=[[-1, 128]],
                        channel_multiplier=1,
                    )

            for h in (hA, hB):
                produce_et(0, h)
                of = None
                for qb in range(nb):
                    if qb + 1 < nb:
                        produce_et(qb + 1, h)
                    # PV: O[q, :64] + denominator in col 64, serial accumulation
                    ob = ob_ps.tile([128, 65], F32, tag="ob")
                    kb0 = max(0, qb - 4)
                    for kb in range(kb0, qb + 1):
                        nc.tensor.matmul(
                            ob[:],
                            et[h][kb][:, (qb - kb) * 128 : (qb - kb) * 128 + 128],
                            vp[h][:, kb, :],
                            start=(kb == kb0),
                            stop=(kb == qb),
                        )
                    if qb >= 4:
                        del et[h][qb - 4]
                    # drain + normalize, batched per 4 query blocks
                    if qb % 4 == 0:
                        of = outs.tile([128, 4 * 65], F32, tag="of", name=f"of_{h}_{qb}")
                    nc.vector.tensor_copy(of[:, (qb % 4) * 65 : (qb % 4) * 65 + 65], ob[:])
                    if qb % 4 == 3 or qb == nb - 1:
                        g = qb // 4
                        nq = qb % 4 + 1
                        rc = outs.tile([128, 4], F32, tag="rc")
                        of3 = of[:].rearrange("p (b c) -> p b c", c=65)
                        nc.vector.reciprocal(rc[:, 0:nq], of3[:, 0:nq, 64])
                        oo = outs.tile([128, 4 * 64], F32, tag="oo")
                        nc.vector.tensor_mul(
                            oo[:, 0 : nq * 64].rearrange("p (b c) -> p b c", c=64),
                            of3[:, 0:nq, 0:64],
                            rc[:, 0:nq].rearrange("p (b c) -> p b c", c=1).broadcast_to(
                                [128, nq, 64]
                            ),
                        )
                        nc.sync.dma_start(
                            out=o_ext[
                                h, g * 512 : g * 512 + nq * 128, :
                            ].rearrange("(b p) d -> p b d", p=128),
                            in_=oo[:, 0 : nq * 64].rearrange(
                                "p (b c) -> p b c", c=64
                            ),
                        )

    nc.compile()
    return nc


_NC_CACHE = {}
TRACE = False
TRACE_DIR = None
LAST_RESULT = None


def _get_nc():
    key = (T, HEADS_PER_CORE)
    if key not in _NC_CACHE:
        _NC_CACHE[key] = build_nc()
    return _NC_CACHE[key]


def kernel(q, k, v):
    q = np.ascontiguousarray(np.asarray(q, dtype=np.float32))
    k = np.ascontiguousarray(np.asarray(k, dtype=np.float32))
    v = np.ascontiguousarray(np.asarray(v, dtype=np.float32))
    assert q.shape == (B, H, T, D)

    qf = q.reshape(B * H, T, D)
    kf = k.reshape(B * H, T, D)
    vf = v.reshape(B * H, T, D)
    ident = np.eye(128, dtype=np.float32)

    in_maps = []
    for c in range(N_CORES):
        s = slice(c * HEADS_PER_CORE, (c + 1) * HEADS_PER_CORE)
        in_maps.append(
            {
                "q": np.ascontiguousarray(qf[s]),
                "k": np.ascontiguousarray(kf[s]),
                "v": np.ascontiguousarray(vf[s]),
                "ident": ident,
            }
        )

    nc = _get_nc()
    global LAST_RESULT
    res = run_bass_kernel_spmd(
        nc, in_maps, list(range(N_CORES)), trace=TRACE, tmpdir=TRACE_DIR
    )
    LAST_RESULT = res
    out = np.concatenate([res.results[c]["out"] for c in range(N_CORES)], axis=0)
    return out.reshape(B, H, T, D).astype(np.float32)
